# revision 12
# baseline (speedup 1.0000x reference)
"""CRY gate kernel for Trainium2 (raw Bass/Bacc), 8-core SPMD.

The reference builds a sparse 4096x4096 complex unitary U for a controlled-RY
gate (control = wire 0 = MSB, target = wire 1) and computes U @ x.  The gate
structure collapses to:

    rows [0, 2048)          : identity
    rows A=[2048, 3072) and B=[3072, 4096), paired r <-> r+1024:
        yA =  c*A - s*B
        yB = -s*A + c*B        with c = cos(theta/2), s = sin(theta/2)

applied independently to the real and imaginary parts (U is real).

Sharding: data-parallel over the batch 128 -> 16 columns per core; theta is
replicated and sin/cos are computed on-device on the Vector engine
(magic-number round + odd minimax polynomial for sin(2*pi*f)).

Raw Bacc (no TileContext) to avoid the Tile kernel-tail drain/barrier
butterfly.  DMA work is packet-bound (~150-250ns per packet on a DMA engine),
so the rotation block is laid out on 32 partitions: partition p holds A-rows
2048+32p..+31 in cols 0:512 and B-rows 3072+32p..+31 in cols 512:1024 -- 2KB
contiguous DRAM per partition per block, 64 packets per 128KB transfer
(vs 256 at 128 partitions).  Queues:

    gpsimd: yr/yi[0:2048] <- xr/xi[0:2048] DRAM->DRAM; tail semaphore clear
    sync  : xr[2048:4096] -> Xr; Xr -> yr[2048:4096]
    scalar: theta -> SBUF (32-partition bcast, tiny); xi -> Xi; Xi -> yi
    vector: sin/cos chain, then per component:
              P = s * [B|A]   (two half-width tensor_scalar ops)
              X <- (X * c) - P   (one fused scalar_tensor_tensor, in place)

Same-engine RAW hazards on the pipelined DVE are ordered with engine-local
DRAINs (cheaper than semaphore round-trips); cross-engine edges use
semaphores.  The kernel clears its semaphores at the end (behind one light
all-engine barrier) so repeated NEFF executions see a clean state.
"""

import sys

import numpy as np

for _p in ("/opt/trn_rl_repo",):
    if _p not in sys.path:
        sys.path.insert(0, _p)

D = 4096
BATCH = 128
NCORES = 8
BL = BATCH // NCORES  # 16 columns per core
NP = 32  # partitions used for the rotation block (2KB DMA packets)
H = 2048  # identity rows
Q = 1024  # rotation block size
FA = Q * BL // NP  # free-dim per component block = 512

# sin(2*pi*f) = f * sum_k KSIN[k] * (f^2)^k  for f in [-0.5, 0.5]  (deg 4,
# max abs err ~6e-6)
KSIN = [
    6.283054082191078,
    -41.331122580391586,
    81.36549238026443,
    -74.47093984475363,
    32.76882701641142,
]
MAGIC = 12582912.0  # 1.5 * 2^23: (x + MAGIC) - MAGIC == round(x) in fp32
INV_4PI = float(1.0 / (4.0 * np.pi))

_STATE: dict = {}


def _drop_const_ap_memsets(nc):
    """The Bass preamble memsets four const-AP tiles this kernel never uses;
    they are the first profiler-"useful" ops and start the measured clock
    ~0.8us before any real work.  Drop them if the module structure allows."""
    try:
        for func in nc.m.functions:
            for block in func.blocks:
                for bb in block.bbs:
                    keep = []
                    for inst in bb.instructions:
                        drop = (
                            inst.__class__.__name__ == "InstMemset"
                            and any(
                                "const-" in str(getattr(o, "memsetref", ""))
                                or "const-" in str(o)
                                for o in inst.outs
                            )
                        )
                        if not drop:
                            keep.append(inst)
                    if len(keep) != len(bb.instructions):
                        bb.instructions[:] = keep
    except Exception:
        pass  # cosmetic optimization only


def _build_nc():
    import concourse.bacc as bacc
    import concourse.mybir as mybir

    f32 = mybir.dt.float32
    mult = mybir.AluOpType.mult
    add = mybir.AluOpType.add
    sub = mybir.AluOpType.subtract

    nc = bacc.Bacc("TRN2", target_bir_lowering=False, debug=False)
    xr = nc.dram_tensor("xr", [D, BL], f32, kind="ExternalInput").ap()
    xi = nc.dram_tensor("xi", [D, BL], f32, kind="ExternalInput").ap()
    th = nc.dram_tensor("th", [1], f32, kind="ExternalInput").ap()
    yr = nc.dram_tensor("yr", [D, BL], f32, kind="ExternalOutput").ap()
    yi = nc.dram_tensor("yi", [D, BL], f32, kind="ExternalOutput").ap()

    def pairs(t):
        # rows [H, D) as [32, 2, 512]: [:, 0, :] = A rows, [:, 1, :] = B rows
        return t[H:D, :].rearrange("(h p r) c -> p h (r c)", h=2, p=NP)

    def halves(t):
        # matching [32, 2, 512] view of a [32, 1024] SBUF tile
        return t.rearrange("p (h f) -> p h f", h=2)

    # SBUF tiles (persistent allocations)
    thb = nc.alloc_sbuf_tensor("thb", [NP, 1], f32).ap()
    v2 = nc.alloc_sbuf_tensor("v2", [NP, 2], f32).ap()
    t1 = nc.alloc_sbuf_tensor("t1", [NP, 2], f32).ap()
    r1 = nc.alloc_sbuf_tensor("r1", [NP, 2], f32).ap()
    g = nc.alloc_sbuf_tensor("g", [NP, 2], f32).ap()
    z = nc.alloc_sbuf_tensor("z", [NP, 2], f32).ap()
    p0 = nc.alloc_sbuf_tensor("p0", [NP, 2], f32).ap()
    p1 = nc.alloc_sbuf_tensor("p1", [NP, 2], f32).ap()
    sc = nc.alloc_sbuf_tensor("sc", [NP, 2], f32).ap()
    Xr = nc.alloc_sbuf_tensor("Xr", [NP, 2 * FA], f32).ap()
    Xi = nc.alloc_sbuf_tensor("Xi", [NP, 2 * FA], f32).ap()
    Pr = nc.alloc_sbuf_tensor("Pr", [NP, 2 * FA], f32).ap()
    Pi = nc.alloc_sbuf_tensor("Pi", [NP, 2 * FA], f32).ap()

    # semaphores (contiguous range right after bass's built-ins)
    sems = [nc.alloc_semaphore(n) for n in (
        "th_sem", "ldr_sem", "ldi_sem", "dve_r", "dve_i",
        "str_sem", "sti_sem", "d2d_sem",
    )]
    th_sem, ldr_sem, ldi_sem, dve_r, dve_i, str_sem, sti_sem, d2d_sem = sems
    sem_lo = min(s.num for s in sems)
    sem_hi = max(s.num for s in sems)
    assert sem_hi - sem_lo + 1 == len(sems), [s.num for s in sems]

    # --- GpSimd: identity rows (DRAM->DRAM, no deps) ---
    nc.gpsimd.dma_start(out=yr[0:H, :], in_=xr[0:H, :]).then_inc(d2d_sem, 16)
    nc.gpsimd.dma_start(out=yi[0:H, :], in_=xi[0:H, :]).then_inc(d2d_sem, 16)

    # --- Sync sequencer: xr load, yr store ---
    nc.sync.dma_start(out=halves(Xr), in_=pairs(xr)).then_inc(ldr_sem, 16)
    nc.sync.wait_ge(dve_r, 1)  # Xr rotation done (implies load consumed)
    nc.sync.dma_start(out=pairs(yr), in_=halves(Xr)).then_inc(str_sem, 16)

    # --- Scalar sequencer: theta bcast (tiny, first), xi load, yi store ---
    nc.scalar.dma_start(out=thb, in_=th.to_broadcast((NP, 1))).then_inc(th_sem, 16)
    nc.scalar.dma_start(out=halves(Xi), in_=pairs(xi)).then_inc(ldi_sem, 16)
    nc.scalar.wait_ge(dve_i, 1)  # Xi rotation done
    nc.scalar.dma_start(out=pairs(yi), in_=halves(Xi)).then_inc(sti_sem, 16)

    # --- Vector engine: sin/cos chain + rotations; same-engine RAW via DRAIN
    V = nc.vector

    V.wait_ge(th_sem, 16)
    # lanes {v, v+0.25} with v = theta/(4*pi)  (no const tile needed)
    V.tensor_scalar(v2[:, 0:1], thb, INV_4PI, None, mult)
    V.tensor_scalar(v2[:, 1:2], thb, INV_4PI, 0.25, mult, add)
    V.drain()
    V.tensor_scalar(t1, v2, MAGIC, None, add)
    V.drain()
    V.tensor_scalar(r1, t1, MAGIC, None, sub)  # round(v2)
    V.drain()
    V.tensor_sub(g, v2, r1)  # wrapped to [-0.5, 0.5]
    V.drain()
    V.tensor_mul(z, g, g)
    V.drain()
    V.tensor_scalar(p0, z, KSIN[4], KSIN[3], mult, add)
    V.drain()
    for kk in (KSIN[2], KSIN[1], KSIN[0]):
        V.tensor_mul(p1, p0, z)
        V.drain()
        V.tensor_scalar(p0, p1, kk, None, add)
        V.drain()
    V.tensor_mul(sc, p0, g)  # lanes {sin(th/2), cos(th/2)}
    V.drain()
    s_ap = sc[:, 0:1]
    c_ap = sc[:, 1:2]

    V.wait_ge(ldr_sem, 16)
    V.tensor_scalar(Pr[:, 0:FA], Xr[:, FA : 2 * FA], s_ap, None, mult)  # s*B
    V.tensor_scalar(Pr[:, FA : 2 * FA], Xr[:, 0:FA], s_ap, None, mult)  # s*A
    V.drain()
    V.scalar_tensor_tensor(Xr, Xr, c_ap, Pr, mult, sub).then_inc(dve_r, 1)
    V.wait_ge(ldi_sem, 16)
    V.tensor_scalar(Pi[:, 0:FA], Xi[:, FA : 2 * FA], s_ap, None, mult)
    V.tensor_scalar(Pi[:, FA : 2 * FA], Xi[:, 0:FA], s_ap, None, mult)
    V.drain()
    V.scalar_tensor_tensor(Xi, Xi, c_ap, Pi, mult, sub).then_inc(dve_i, 1)

    # --- GpSimd tail: wait for every completion, clear our semaphores ---
    nc.gpsimd.wait_ge(th_sem, 16)
    nc.gpsimd.wait_ge(ldr_sem, 16)
    nc.gpsimd.wait_ge(ldi_sem, 16)
    nc.gpsimd.wait_ge(dve_r, 1)
    nc.gpsimd.wait_ge(dve_i, 1)
    nc.gpsimd.wait_ge(str_sem, 16)
    nc.gpsimd.wait_ge(sti_sem, 16)
    nc.gpsimd.wait_ge(d2d_sem, 32)
    # one light barrier so the clear is globally ordered (the dedicated
    # barrier sems return to 0 by design, so they need no clearing)
    nc.all_engine_barrier()
    nc.gpsimd.sem_clear(range(sem_lo, sem_hi + 1))

    _drop_const_ap_memsets(nc)
    nc.compile()
    return nc


def _get_nc():
    if "nc" not in _STATE:
        _STATE["nc"] = _build_nc()
    return _STATE["nc"]


def _run(xr, xi, th, **kwargs):
    """Run the SPMD kernel on 8 cores. Returns (y_complex, BassKernelResults)."""
    from concourse.bass_utils import run_bass_kernel_spmd

    nc = _get_nc()
    in_maps = [
        {
            "xr": np.ascontiguousarray(xr[:, k * BL : (k + 1) * BL]),
            "xi": np.ascontiguousarray(xi[:, k * BL : (k + 1) * BL]),
            "th": th,
        }
        for k in range(NCORES)
    ]
    out = run_bass_kernel_spmd(nc, in_maps, list(range(NCORES)), **kwargs)
    yr = np.concatenate([out.results[k]["yr"] for k in range(NCORES)], axis=1)
    yi = np.concatenate([out.results[k]["yi"] for k in range(NCORES)], axis=1)
    y = yr.astype(np.complex64)
    y.imag = yi
    return y, out


def kernel(x_real, x_imag, theta):
    xr = np.ascontiguousarray(np.asarray(x_real, dtype=np.float32))
    xi = np.ascontiguousarray(np.asarray(x_imag, dtype=np.float32))
    th = np.ascontiguousarray(np.asarray(theta, dtype=np.float32)).reshape(1)
    y, _ = _run(xr, xi, th)
    return y
